# revision 15
# baseline (speedup 1.0000x reference)
"""Trainium2 Bass kernel for the stacked-Chebyshev locally-connected net.

Reference computation (B=256, k=6250, d*d=4096, O=10):
    x1 = z @ (mask*T1).T
    x2 = 2*(z @ (mask*T2).T)*x1 - T0
    x3 = 2*(z @ (mask*T3).T)*x2 - x1
    out = x3 @ C_w.T + C_b

The mask is a locally-connected conv pattern: 16x16 patch, stride 2, 25x25
positions, stacked 10x.  Rows sharing patch-row i and a patch-column BLOCK
(j-range) share a 16-row x 32-col window of the image -- 512 contraction
entries instead of the 1024-wide full-row window, which halves both the
weight bytes streamed from HBM and the matmul count vs the row-only
grouping.

Per core: 3 whole patch-row groups (i = 3c .. 3c+2), each split into three
j-blocks {0-8}, {9-16}, {17-24} (90/80/80 k-columns).  z is staged per
column band (cols [0,32), [18,50), [32,64)) as a 640-entry array of the
core's 20 image rows; group Delta=0/2 windows are 128-aligned slices (4
chunks), the Delta=1 window is covered with 5 chunks whose first/last
weight pieces are zero-padded half-chunks.  Group 24 (the 25th) is split
across cores by j-bands (3-4 positions, 352-entry window, 3 chunks).

The Chebyshev recurrence is elementwise in [k, B]; its five ops per unit
are spread over three engines (ACT: x1 copy + T0 bias add, DVE: the two
products, GPSIMD: the x3 subtract) so no single engine paces the PE.  The
k->O projection accumulates in PSUM per core; per-core partials are summed
on the host.  A burst of dummy matmuls on memset data warms the PE clock
(HAM) before the first weights arrive.
"""

import ml_dtypes
import numpy as np

import concourse.bass as bass
import concourse.mybir as mybir
import concourse.tile as tile
from concourse import bacc
from concourse.bass_utils import run_bass_kernel_spmd

F32 = mybir.dt.float32
F32R = mybir.dt.float32r

B = 256          # batch
O = 10           # output classes
D = 64           # image side
N_CORES = 8
N_GROUPS = 3     # whole patch-row groups per core
MM_MODE = "f16"
MM_DT = {"f16": mybir.dt.float16, "bf16": mybir.dt.bfloat16,
         "f32r": F32R}[MM_MODE]

# j-blocks within a group and their z column bands
BLK_J = (range(0, 9), range(9, 17), range(17, 25))
BLK_C0 = (0, 18, 32)          # band start column
BLK_COLS = (90, 80, 80)       # 10 * len(j-block)
BW = 32                       # band width
ZB_CH = 5                     # chunks per band array (20 rows * 32 = 640)
# chunk lists per group Delta (window = band entries [64*Delta, 64*Delta+512))
CHUNKS = ((0, 1, 2, 3), (0, 1, 2, 3, 4), (1, 2, 3, 4))

# mini: group 24 split by j-bands across cores
MINI_J0 = (0, 3, 6, 9, 12, 15, 18, 21)
MINI_NJ = (3, 3, 3, 3, 3, 3, 3, 4)
MINI_W = 22                   # padded window width (2*4+14)
MINI_CH = 3                   # ceil(16*22/128)
MINI_COLS = 40                # padded k-columns per core

N_UNITS = 3 * N_GROUPS + 1    # 9 full units + 1 mini
W_COLS_G = [len(CHUNKS[d]) * sum(BLK_COLS) for d in range(3)]  # 1000/1250/1000


def _build_nc():
    nc = bacc.Bacc(
        "TRN2", target_bir_lowering=False, debug=False, num_devices=N_CORES
    )
    # banded z: 3 band arrays of 5 chunks each, chunk-blocked on the host
    zb = nc.dram_tensor("zb", [3 * ZB_CH * 128, B], MM_DT,
                        kind="ExternalInput").ap()
    # per-layer packed weight pieces, columns in (group, block, piece) order
    w_dram = [
        nc.dram_tensor(f"w{l}", [128, sum(W_COLS_G)], MM_DT,
                       kind="ExternalInput").ap()
        for l in (1, 2, 3)
    ]
    # mini z window (3 chunks) and weights (layer-major, then chunk)
    zg = nc.dram_tensor("zg", [MINI_CH * 128, B], MM_DT,
                        kind="ExternalInput").ap()
    wm_dram = nc.dram_tensor("wm", [128, 3 * MINI_CH * MINI_COLS], MM_DT,
                             kind="ExternalInput").ap()
    t0n = nc.dram_tensor("t0n", [128, N_UNITS], F32, kind="ExternalInput").ap()
    cwt = nc.dram_tensor("cwt", [128, N_UNITS * O], F32R,
                         kind="ExternalInput").ap()
    out = nc.dram_tensor("out", [O, B], F32, kind="ExternalOutput").ap()

    # weight DRAM column offsets per (group, layer-agnostic block piece set)
    w_off = {}
    col = 0
    for g in range(N_GROUPS):
        for b in range(3):
            w_off[(g, b)] = col
            col += len(CHUNKS[g]) * BLK_COLS[b]
    assert col == sum(W_COLS_G)

    with tile.TileContext(nc) as tc:
        with (
            tc.tile_pool(name="zpool", bufs=1) as zpool,
            tc.tile_pool(name="cpool", bufs=1) as cpool,
            tc.tile_pool(name="wpool", bufs=12) as wpool,
            tc.tile_pool(name="xpool", bufs=8) as xpool,
            tc.tile_pool(name="ppool", bufs=7, space="PSUM") as ppool,
            tc.tile_pool(name="opool", bufs=1, space="PSUM") as opool,
        ):
            engs = (nc.sync, nc.scalar)
            n_dma = 0

            def dma(dst, src):
                nonlocal n_dma
                engs[n_dma % 2].dma_start(dst, src)
                n_dma += 1

            # HAM warm-up: dummy matmuls on memset data keep the PE busy
            # from the start so its clock unthrottles while weights stream.
            dummy_sb = zpool.tile([128, B], MM_DT, tag="warm")
            nc.gpsimd.memset(dummy_sb[:], 0)
            dummy_ps = ppool.tile([128, B], F32, tag="ps")

            def warm(n):
                for _ in range(n):
                    nc.tensor.matmul(dummy_ps[:], dummy_sb[:, 0:128],
                                     dummy_sb[:], start=True, stop=True)

            warm(12)

            # z band tiles, chunk-blocked [128, ZB_CH*B]
            zbt = [zpool.tile([128, ZB_CH * B], MM_DT, tag=f"zb{b}",
                              name=f"zbt{b}")
                   for b in range(3)]

            def zb_dma(b, c0, c1, eng):
                eng.dma_start(
                    zbt[b][:, c0 * B:c1 * B].rearrange("p (c n) -> p c n", n=B),
                    zb[(b * ZB_CH + c0) * 128:(b * ZB_CH + c1) * 128, :]
                    .rearrange("(c p) n -> p c n", p=128))

            # weight tile for one (group, layer, block): all its pieces
            def wload(g, li, b, per_chunk=False):
                np_ = len(CHUNKS[g])
                m = BLK_COLS[b]
                t = wpool.tile([128, np_ * m], MM_DT, tag="w")
                c0 = w_off[(g, b)]
                if per_chunk:
                    for p in range(np_):
                        engs[p % 2].dma_start(
                            t[:, p * m:(p + 1) * m],
                            w_dram[li][:, c0 + p * m:c0 + (p + 1) * m])
                else:
                    dma(t[:], w_dram[li][:, c0:c0 + np_ * m])
                return t

            # critical-path prefix: first block's weights per chunk on one
            # queue, its z chunks on the other, so matmul #0 waits ~90 KB
            wt = {}
            wt[(0, 0, 0)] = wload(0, 0, 0, per_chunk=True)
            for c in range(4):
                zb_dma(0, c, c + 1, nc.scalar)
            wt[(0, 0, 1)] = wload(0, 0, 1)
            zb_dma(1, 0, 2, nc.scalar)
            wt[(0, 0, 2)] = wload(0, 0, 2)
            zb_dma(1, 2, 4, nc.scalar)
            for li in (1, 2):
                for b in range(3):
                    wt[(0, li, b)] = wload(0, li, b)
            zb_dma(2, 0, 2, nc.sync)
            zb_dma(2, 2, 4, nc.scalar)

            t0_sb = cpool.tile([128, N_UNITS], F32, tag="t0")
            cw_sb = cpool.tile([128, N_UNITS * O], F32R, tag="cw")
            dma(t0_sb[:], t0n[:])
            dma(cw_sb[:], cwt[:])

            for li in range(3):
                for b in range(3):
                    wt[(1, li, b)] = wload(1, li, b)
            zb_dma(0, 4, 5, nc.sync)
            zb_dma(1, 4, 5, nc.scalar)
            zb_dma(2, 4, 5, nc.sync)

            # mini data
            zgt = zpool.tile([128, MINI_CH * B], MM_DT, tag="zg")
            nc.scalar.dma_start(
                zgt[:].rearrange("p (c n) -> p c n", n=B),
                zg[:].rearrange("(c p) n -> p c n", p=128))
            wm = wpool.tile([128, 3 * MINI_CH * MINI_COLS], MM_DT, tag="wm")
            dma(wm[:], wm_dram[:])

            psum_o = opool.tile([O, B], F32)
            n_proj = 0
            pending = []   # deferred projection matmuls (src, unit, rows)

            def project(src_t, unit, rows):
                pending.append((src_t, unit, rows))

            def flush_proj():
                nonlocal n_proj
                for src_t, unit, rows in pending:
                    n_proj += 1
                    nc.tensor.matmul(psum_o[:],
                                     cw_sb[0:rows, unit * O:(unit + 1) * O],
                                     src_t[:],
                                     start=(n_proj == 1),
                                     stop=(n_proj == N_UNITS))
                pending.clear()

            def recurrence(li, p, xs, unit, rows):
                """Chebyshev step for one unit; ops spread over ACT/DVE/GP."""
                if li == 0:
                    x1 = xpool.tile([rows, B], F32, tag="x1")
                    nc.scalar.copy(x1[:], p[:])
                    xs["x1"] = x1
                elif li == 1:
                    m2 = xpool.tile([rows, B], F32, tag="m2")
                    x2 = xpool.tile([rows, B], F32, tag="x2")
                    nc.vector.tensor_mul(m2[:], p[:], xs["x1"][:])
                    nc.scalar.add(x2[:], m2[:], t0_sb[0:rows, unit:unit + 1])
                    xs["x2"] = x2
                else:
                    m3 = xpool.tile([rows, B], F32, tag="m3")
                    x3 = xpool.tile([rows, B], F32R, tag="x3")
                    nc.vector.tensor_mul(m3[:], p[:], xs["x2"][:])
                    nc.gpsimd.tensor_sub(x3[:], m3[:], xs["x1"][:])
                    project(x3, unit, rows)

            def group(g, fill=0):
                units = [{}, {}, {}]
                chunks = CHUNKS[g]
                for li in range(3):
                    flush_proj()
                    for b in range(3):
                        m = BLK_COLS[b]
                        w = wt[(g, li, b)]
                        p = ppool.tile([m, B], F32, tag="ps")
                        for pi, kc in enumerate(chunks):
                            nc.tensor.matmul(
                                p[:], w[:, pi * m:(pi + 1) * m],
                                zbt[b][:, kc * B:(kc + 1) * B],
                                start=(pi == 0), stop=(pi == len(chunks) - 1))
                            if li == 0:
                                warm(fill)
                        recurrence(li, p, units[b], 3 * g + b, m)

            group(0, fill=1)

            for li in range(3):
                for b in range(3):
                    wt[(2, li, b)] = wload(2, li, b)

            group(1)

            # mini between the two remaining groups to cover their DMA
            mini = {}
            for li in range(3):
                flush_proj()
                p = ppool.tile([MINI_COLS, B], F32, tag="ps")
                for kc in range(MINI_CH):
                    lhsT = wm[:, (li * MINI_CH + kc) * MINI_COLS:
                              (li * MINI_CH + kc + 1) * MINI_COLS]
                    nc.tensor.matmul(p[:], lhsT, zgt[:, kc * B:(kc + 1) * B],
                                     start=(kc == 0), stop=(kc == MINI_CH - 1))
                recurrence(li, p, mini, N_UNITS - 1, MINI_COLS)

            group(2)
            flush_proj()

            out_sb = cpool.tile([O, B], F32, tag="out")
            nc.scalar.copy(out_sb[:], psum_o[:])
            nc.sync.dma_start(out[:], out_sb[:])

    nc.compile()
    return nc


_NC = None


def _get_nc():
    global _NC
    if _NC is None:
        _NC = _build_nc()
    return _NC


def _win_idx(r0, c0, rows, width):
    """d*d indices of a rows x width window flattened row-major."""
    e = np.arange(rows * width)
    return (r0 + e // width) * D + c0 + e % width


def _prepare_in_maps(z, T1, T2, T3, T0, C_w, mask):
    z = np.ascontiguousarray(np.asarray(z, dtype=np.float32).reshape(B, D * D))
    T1 = np.asarray(T1, dtype=np.float32)
    T2 = np.asarray(T2, dtype=np.float32)
    T3 = np.asarray(T3, dtype=np.float32)
    T0 = np.asarray(T0, dtype=np.float32)
    C_w = np.asarray(C_w, dtype=np.float32)
    mask = np.asarray(mask, dtype=np.float32)

    np_mm = {"f16": np.float16, "bf16": ml_dtypes.bfloat16,
             "f32r": np.float32}[MM_MODE]
    zT = np.ascontiguousarray(z.T)                   # [4096, 256]
    A = [sc * T * mask for T, sc in ((T1, 1.0), (T2, 2.0), (T3, 2.0))]

    in_maps = []
    for c in range(N_CORES):
        i0 = 3 * c
        r0 = 2 * i0
        m = {}

        # banded z: per band, 640 entries = 20 rows x 32 cols
        zb = np.empty((3 * ZB_CH * 128, B), np.float32)
        for b in range(3):
            idx = _win_idx(r0, BLK_C0[b], 20, BW)
            zb[b * ZB_CH * 128:(b + 1) * ZB_CH * 128] = zT[idx]
        m["zb"] = zb.astype(np_mm)

        # weights: per layer, pieces in (group, block, piece) column order
        t0n = np.zeros((128, N_UNITS), np.float32)
        cwt = np.zeros((128, N_UNITS * O), np.float32)
        for l in range(3):
            wl = np.zeros((128, sum(W_COLS_G)), np.float32)
            col = 0
            for g in range(N_GROUPS):
                gg = i0 + g
                chunks = CHUNKS[g]
                for b in range(3):
                    cols_k = np.array([s * 625 + gg * 25 + j
                                       for s in range(10) for j in BLK_J[b]])
                    win = _win_idx(2 * gg, BLK_C0[b], 16, BW)
                    AT = A[l][np.ix_(cols_k, win)].T      # [512, m]
                    mcols = len(cols_k)
                    for pi, kc in enumerate(chunks):
                        piece = np.zeros((128, mcols), np.float32)
                        # band entries of chunk kc vs window [64g_, 64g_+512)
                        lo = kc * 128 - 64 * g
                        wlo, whi = max(lo, 0), min(lo + 128, 512)
                        piece[wlo - lo:whi - lo] = AT[wlo:whi]
                        wl[:, col:col + mcols] = piece
                        col += mcols
                    if l == 0:
                        u = 3 * g + b
                        t0n[0:mcols, u] = -T0[cols_k]
                        cwt[0:mcols, u * O:(u + 1) * O] = C_w[:, cols_k].T
            m[f"w{l + 1}"] = np.ascontiguousarray(wl).astype(np_mm)

        # mini: group 24, this core's j-band
        j0, nj = MINI_J0[c], MINI_NJ[c]
        mcols_k = np.array([s * 625 + 600 + j
                            for s in range(10) for j in range(j0, j0 + nj)])
        nmc = len(mcols_k)
        gwin = _win_idx(48, 2 * j0, 16, MINI_W)          # 352 entries
        zgm = np.zeros((MINI_CH * 128, B), np.float32)
        zgm[0:352] = zT[gwin]
        m["zg"] = zgm.astype(np_mm)
        wm = np.zeros((128, 3 * MINI_CH * MINI_COLS), np.float32)
        for l in range(3):
            AT = A[l][np.ix_(mcols_k, gwin)].T           # [352, nmc]
            for kc in range(MINI_CH):
                piece = np.zeros((128, MINI_COLS), np.float32)
                hi = min(352 - kc * 128, 128)
                piece[0:hi, 0:nmc] = AT[kc * 128:kc * 128 + hi]
                wm[:, (l * MINI_CH + kc) * MINI_COLS:
                   (l * MINI_CH + kc + 1) * MINI_COLS] = piece
        m["wm"] = wm.astype(np_mm)
        u = N_UNITS - 1
        t0n[0:nmc, u] = -T0[mcols_k]
        cwt[0:nmc, u * O:(u + 1) * O] = C_w[:, mcols_k].T
        m["t0n"] = t0n
        m["cwt"] = cwt
        in_maps.append(m)
    return in_maps


def kernel(z, T1, T2, T3, T0, C_w, C_b, mask):
    nc = _get_nc()
    in_maps = _prepare_in_maps(z, T1, T2, T3, T0, C_w, mask)
    res = run_bass_kernel_spmd(nc, in_maps, core_ids=list(range(N_CORES)))
    total = np.zeros((O, B), np.float32)
    for c in range(N_CORES):
        total += res.results[c]["out"]
    C_b = np.asarray(C_b, dtype=np.float32)
    return (total.T + C_b).astype(np.float32)


# revision 21
# speedup vs baseline: 1.1969x; 1.1969x over previous
"""Trainium2 Bass kernel for the stacked-Chebyshev locally-connected net.

Reference computation (B=256, k=6250, d*d=4096, O=10):
    x1 = z @ (mask*T1).T
    x2 = 2*(z @ (mask*T2).T)*x1 - T0
    x3 = 2*(z @ (mask*T3).T)*x2 - x1
    out = x3 @ C_w.T + C_b

The mask is a locally-connected conv pattern: 16x16 patch, stride 2, 25x25
positions, stacked 10x.  Rows sharing patch-row i and a patch-column BLOCK
(j-range) share a 16-row x 32-col window of the image -- 512 contraction
entries instead of the 1024-wide full-row window, which halves both the
weight bytes streamed from HBM and the matmul count vs the row-only
grouping.

Per core: 3 whole patch-row groups (i = 3c .. 3c+2), each split into three
j-blocks {0-8}, {9-16}, {17-24} (90/80/80 k-columns).  z is staged per
column band (cols [0,32), [18,50), [32,64)) as a 640-entry array of the
core's 20 image rows; group Delta=0/2 windows are 128-aligned slices (4
chunks), the Delta=1 window is covered with 5 chunks whose first/last
weight pieces are zero-padded half-chunks.  Group 24 (the 25th) is split
across cores by j-bands (3-4 positions, 352-entry window, 3 chunks).

The Chebyshev recurrence is elementwise in [k, B]; its five ops per unit
are spread over three engines (ACT: x1 copy + T0 bias add, DVE: the two
products, GPSIMD: the x3 subtract) so no single engine paces the PE.  The
k->O projection accumulates in PSUM per core; per-core partials are summed
on the host.  A burst of dummy matmuls on memset data warms the PE clock
(HAM) before the first weights arrive.
"""

import ml_dtypes
import numpy as np

import concourse.bass as bass
import concourse.mybir as mybir
import concourse.tile as tile
from concourse import bacc
from concourse.bass_utils import run_bass_kernel_spmd

F32 = mybir.dt.float32
F32R = mybir.dt.float32r

B = 256          # batch
O = 10           # output classes
D = 64           # image side
N_CORES = 8
N_GROUPS = 3     # whole patch-row groups per core
MM_MODE = "f16"
MM_DT = {"f16": mybir.dt.float16, "bf16": mybir.dt.bfloat16,
         "f32r": F32R}[MM_MODE]

# j-blocks within a group and their z column bands
BLK_J = (range(0, 9), range(9, 17), range(17, 25))
BLK_C0 = (0, 18, 32)          # band start column
BLK_COLS = (90, 80, 80)       # 10 * len(j-block)
BW = 32                       # band width
ZB_CH = 5                     # chunks per band array (20 rows * 32 = 640)
# chunk lists per group Delta (window = band entries [64*Delta, 64*Delta+512))
CHUNKS = ((0, 1, 2, 3), (0, 1, 2, 3, 4), (1, 2, 3, 4))

# mini: group 24 split by j-bands across cores
MINI_J0 = (0, 3, 6, 9, 12, 15, 18, 21)
MINI_NJ = (3, 3, 3, 3, 3, 3, 3, 4)
MINI_W = 22                   # padded window width (2*4+14)
MINI_CH = 3                   # ceil(16*22/128)
MINI_COLS = 40                # padded k-columns per core

N_UNITS = 3 * N_GROUPS + 1    # 9 full units + 1 mini
W_COLS_G = [len(CHUNKS[d]) * sum(BLK_COLS) for d in range(3)]  # 1000/1250/1000


def _build_nc():
    nc = bacc.Bacc(
        "TRN2", target_bir_lowering=False, debug=False, num_devices=N_CORES
    )
    # banded z: 3 band arrays of 5 chunks each, chunk-blocked on the host
    zb = nc.dram_tensor("zb", [3 * ZB_CH * 128, B], MM_DT,
                        kind="ExternalInput").ap()
    # one packed weight tensor per (layer, group) so each loads with a
    # single fully-contiguous DMA; columns in (block, piece) order
    w_dram = {
        (li, g): nc.dram_tensor(f"w{li + 1}g{g}", [128, W_COLS_G[g]], MM_DT,
                                kind="ExternalInput").ap()
        for li in range(3) for g in range(N_GROUPS)
    }
    # mini z window (3 chunks) and weights (layer-major, then chunk)
    zg = nc.dram_tensor("zg", [MINI_CH * 128, B], MM_DT,
                        kind="ExternalInput").ap()
    wm_dram = nc.dram_tensor("wm", [128, 3 * MINI_CH * MINI_COLS], MM_DT,
                             kind="ExternalInput").ap()
    t0n = nc.dram_tensor("t0n", [128, N_UNITS], F32, kind="ExternalInput").ap()
    cwt = nc.dram_tensor("cwt", [128, N_UNITS * O], F32R,
                         kind="ExternalInput").ap()
    out = nc.dram_tensor("out", [O, B], F32, kind="ExternalOutput").ap()

    # column offset of block b inside a (layer, group) weight tensor
    def blk_off(g, b):
        return len(CHUNKS[g]) * sum(BLK_COLS[:b])

    with tile.TileContext(nc) as tc:
        with (
            tc.tile_pool(name="zpool", bufs=1) as zpool,
            tc.tile_pool(name="cpool", bufs=1) as cpool,
            tc.tile_pool(name="wpool", bufs=12) as wpool,
            tc.tile_pool(name="xpool", bufs=8) as xpool,
            tc.tile_pool(name="ppool", bufs=7, space="PSUM") as ppool,
            tc.tile_pool(name="opool", bufs=1, space="PSUM") as opool,
        ):
            engs = (nc.sync, nc.scalar)
            n_dma = 0

            def dma(dst, src):
                nonlocal n_dma
                engs[n_dma % 2].dma_start(dst, src)
                n_dma += 1

            # HAM warm-up: dummy matmuls on memset data keep the PE busy
            # from the start so its clock unthrottles while weights stream.
            dummy_sb = zpool.tile([128, B], MM_DT, tag="warm")
            nc.gpsimd.memset(dummy_sb[:], 0)
            dummy_ps = ppool.tile([128, B], F32, tag="ps")

            def warm(n):
                for _ in range(n):
                    nc.tensor.matmul(dummy_ps[:], dummy_sb[:, 0:128],
                                     dummy_sb[:], start=True, stop=True)

            warm(12)

            # z band tiles, chunk-blocked [128, ZB_CH*B]
            zbt = [zpool.tile([128, ZB_CH * B], MM_DT, tag=f"zb{b}",
                              name=f"zbt{b}")
                   for b in range(3)]

            def zb_dma(b, c0, c1, eng):
                eng.dma_start(
                    zbt[b][:, c0 * B:c1 * B].rearrange("p (c n) -> p c n", n=B),
                    zb[(b * ZB_CH + c0) * 128:(b * ZB_CH + c1) * 128, :]
                    .rearrange("(c p) n -> p c n", p=128))

            # weight tile for one (group, layer): all 3 blocks, one
            # contiguous DMA (split in two for the very first tile)
            def wload(g, li, split=False):
                t = wpool.tile([128, W_COLS_G[g]], MM_DT, tag="w")
                if split:
                    half = blk_off(g, 1)
                    nc.sync.dma_start(t[:, 0:half], w_dram[(li, g)][:, 0:half])
                    nc.sync.dma_start(t[:, half:], w_dram[(li, g)][:, half:])
                else:
                    dma(t[:], w_dram[(li, g)][:])
                return t

            # critical-path prefix: first layer-group weights on one queue,
            # its z band chunks on the other
            wt = {}
            wt[(0, 0)] = wload(0, 0, split=True)
            zb_dma(0, 0, 2, nc.scalar)
            zb_dma(0, 2, 4, nc.scalar)
            wt[(0, 1)] = wload(0, 1, split=True)
            zb_dma(1, 0, 2, nc.scalar)
            zb_dma(1, 2, 4, nc.scalar)
            wt[(0, 2)] = wload(0, 2)
            zb_dma(2, 0, 4, nc.scalar)

            t0_sb = cpool.tile([128, N_UNITS], F32, tag="t0")
            cw_sb = cpool.tile([128, N_UNITS * O], F32R, tag="cw")
            dma(t0_sb[:], t0n[:])
            dma(cw_sb[:], cwt[:])

            for li in range(3):
                wt[(1, li)] = wload(1, li)
            zb_dma(0, 4, 5, nc.sync)
            zb_dma(1, 4, 5, nc.scalar)
            zb_dma(2, 4, 5, nc.sync)

            # mini data
            zgt = zpool.tile([128, MINI_CH * B], MM_DT, tag="zg")
            nc.scalar.dma_start(
                zgt[:].rearrange("p (c n) -> p c n", n=B),
                zg[:].rearrange("(c p) n -> p c n", p=128))
            wm = wpool.tile([128, 3 * MINI_CH * MINI_COLS], MM_DT, tag="wm")
            dma(wm[:], wm_dram[:])

            psum_o = opool.tile([O, B], F32)
            n_proj = 0
            pending = []   # deferred projection matmuls (src, unit, rows)

            def project(src_t, unit, rows):
                pending.append((src_t, unit, rows))

            def flush_proj():
                nonlocal n_proj
                for src_t, unit, rows in pending:
                    n_proj += 1
                    nc.tensor.matmul(psum_o[:],
                                     cw_sb[0:rows, unit * O:(unit + 1) * O],
                                     src_t[:],
                                     start=(n_proj == 1),
                                     stop=(n_proj == N_UNITS))
                pending.clear()

            def recurrence(li, p, xs, unit, rows):
                """Chebyshev step for one unit; ops spread over ACT/DVE/GP."""
                if li == 0:
                    x1 = xpool.tile([rows, B], F32, tag="x1")
                    nc.scalar.copy(x1[:], p[:])
                    xs["x1"] = x1
                elif li == 1:
                    m2 = xpool.tile([rows, B], F32, tag="m2")
                    x2 = xpool.tile([rows, B], F32, tag="x2")
                    nc.vector.tensor_mul(m2[:], p[:], xs["x1"][:])
                    nc.scalar.add(x2[:], m2[:], t0_sb[0:rows, unit:unit + 1])
                    xs["x2"] = x2
                else:
                    m3 = xpool.tile([rows, B], F32, tag="m3")
                    x3 = xpool.tile([rows, B], F32R, tag="x3")
                    nc.vector.tensor_mul(m3[:], p[:], xs["x2"][:])
                    nc.gpsimd.tensor_sub(x3[:], m3[:], xs["x1"][:])
                    project(x3, unit, rows)

            def group(g, fill=0):
                units = [{}, {}, {}]
                chunks = CHUNKS[g]
                for li in range(3):
                    if li == 1:
                        flush_proj()
                    for b in range(3):
                        m = BLK_COLS[b]
                        w = wt[(g, li)]
                        o = blk_off(g, b)
                        p = ppool.tile([m, B], F32, tag="ps")
                        for pi, kc in enumerate(chunks):
                            nc.tensor.matmul(
                                p[:], w[:, o + pi * m:o + (pi + 1) * m],
                                zbt[b][:, kc * B:(kc + 1) * B],
                                start=(pi == 0), stop=(pi == len(chunks) - 1))
                            if li == 0:
                                warm(fill)
                        recurrence(li, p, units[b], 3 * g + b, m)

            group(0, fill=1)

            for li in range(3):
                wt[(2, li)] = wload(2, li)

            group(1)

            # mini between the two remaining groups to cover their DMA
            mini = {}
            for li in range(3):
                if li == 1:
                    flush_proj()
                p = ppool.tile([MINI_COLS, B], F32, tag="ps")
                for kc in range(MINI_CH):
                    lhsT = wm[:, (li * MINI_CH + kc) * MINI_COLS:
                              (li * MINI_CH + kc + 1) * MINI_COLS]
                    nc.tensor.matmul(p[:], lhsT, zgt[:, kc * B:(kc + 1) * B],
                                     start=(kc == 0), stop=(kc == MINI_CH - 1))
                recurrence(li, p, mini, N_UNITS - 1, MINI_COLS)

            group(2)
            flush_proj()

            out_sb = cpool.tile([O, B], F32, tag="out")
            nc.scalar.copy(out_sb[:], psum_o[:])
            nc.sync.dma_start(out[:], out_sb[:])

    nc.compile()
    return nc


_NC = None


def _get_nc():
    global _NC
    if _NC is None:
        _NC = _build_nc()
    return _NC


def _win_idx(r0, c0, rows, width):
    """d*d indices of a rows x width window flattened row-major."""
    e = np.arange(rows * width)
    return (r0 + e // width) * D + c0 + e % width


def _prepare_in_maps(z, T1, T2, T3, T0, C_w, mask):
    z = np.ascontiguousarray(np.asarray(z, dtype=np.float32).reshape(B, D * D))
    T1 = np.asarray(T1, dtype=np.float32)
    T2 = np.asarray(T2, dtype=np.float32)
    T3 = np.asarray(T3, dtype=np.float32)
    T0 = np.asarray(T0, dtype=np.float32)
    C_w = np.asarray(C_w, dtype=np.float32)
    mask = np.asarray(mask, dtype=np.float32)

    np_mm = {"f16": np.float16, "bf16": ml_dtypes.bfloat16,
             "f32r": np.float32}[MM_MODE]
    zT = np.ascontiguousarray(z.T)                   # [4096, 256]
    A = [sc * T * mask for T, sc in ((T1, 1.0), (T2, 2.0), (T3, 2.0))]

    in_maps = []
    for c in range(N_CORES):
        i0 = 3 * c
        r0 = 2 * i0
        m = {}

        # banded z: per band, 640 entries = 20 rows x 32 cols
        zb = np.empty((3 * ZB_CH * 128, B), np.float32)
        for b in range(3):
            idx = _win_idx(r0, BLK_C0[b], 20, BW)
            zb[b * ZB_CH * 128:(b + 1) * ZB_CH * 128] = zT[idx]
        m["zb"] = zb.astype(np_mm)

        # weights: one tensor per (layer, group), pieces in (block, piece)
        # column order so the kernel loads each with one contiguous DMA
        t0n = np.zeros((128, N_UNITS), np.float32)
        cwt = np.zeros((128, N_UNITS * O), np.float32)
        for l in range(3):
            for g in range(N_GROUPS):
                gg = i0 + g
                chunks = CHUNKS[g]
                wl = np.zeros((128, W_COLS_G[g]), np.float32)
                col = 0
                for b in range(3):
                    cols_k = np.array([s * 625 + gg * 25 + j
                                       for s in range(10) for j in BLK_J[b]])
                    win = _win_idx(2 * gg, BLK_C0[b], 16, BW)
                    AT = A[l][np.ix_(cols_k, win)].T      # [512, m]
                    mcols = len(cols_k)
                    for pi, kc in enumerate(chunks):
                        piece = np.zeros((128, mcols), np.float32)
                        # band entries of chunk kc vs window [64g, 64g+512)
                        lo = kc * 128 - 64 * g
                        wlo, whi = max(lo, 0), min(lo + 128, 512)
                        piece[wlo - lo:whi - lo] = AT[wlo:whi]
                        wl[:, col:col + mcols] = piece
                        col += mcols
                    if l == 0:
                        u = 3 * g + b
                        t0n[0:mcols, u] = -T0[cols_k]
                        cwt[0:mcols, u * O:(u + 1) * O] = C_w[:, cols_k].T
                m[f"w{l + 1}g{g}"] = np.ascontiguousarray(wl).astype(np_mm)

        # mini: group 24, this core's j-band
        j0, nj = MINI_J0[c], MINI_NJ[c]
        mcols_k = np.array([s * 625 + 600 + j
                            for s in range(10) for j in range(j0, j0 + nj)])
        nmc = len(mcols_k)
        gwin = _win_idx(48, 2 * j0, 16, MINI_W)          # 352 entries
        zgm = np.zeros((MINI_CH * 128, B), np.float32)
        zgm[0:352] = zT[gwin]
        m["zg"] = zgm.astype(np_mm)
        wm = np.zeros((128, 3 * MINI_CH * MINI_COLS), np.float32)
        for l in range(3):
            AT = A[l][np.ix_(mcols_k, gwin)].T           # [352, nmc]
            for kc in range(MINI_CH):
                piece = np.zeros((128, MINI_COLS), np.float32)
                hi = min(352 - kc * 128, 128)
                piece[0:hi, 0:nmc] = AT[kc * 128:kc * 128 + hi]
                wm[:, (l * MINI_CH + kc) * MINI_COLS:
                   (l * MINI_CH + kc + 1) * MINI_COLS] = piece
        m["wm"] = wm.astype(np_mm)
        u = N_UNITS - 1
        t0n[0:nmc, u] = -T0[mcols_k]
        cwt[0:nmc, u * O:(u + 1) * O] = C_w[:, mcols_k].T
        m["t0n"] = t0n
        m["cwt"] = cwt
        in_maps.append(m)
    return in_maps


def kernel(z, T1, T2, T3, T0, C_w, C_b, mask):
    nc = _get_nc()
    in_maps = _prepare_in_maps(z, T1, T2, T3, T0, C_w, mask)
    res = run_bass_kernel_spmd(nc, in_maps, core_ids=list(range(N_CORES)))
    total = np.zeros((O, B), np.float32)
    for c in range(N_CORES):
        total += res.results[c]["out"]
    C_b = np.asarray(C_b, dtype=np.float32)
    return (total.T + C_b).astype(np.float32)


# revision 28
# speedup vs baseline: 1.2102x; 1.0111x over previous
"""Trainium2 Bass kernel for the stacked-Chebyshev locally-connected net.

Reference computation (B=256, k=6250, d*d=4096, O=10):
    x1 = z @ (mask*T1).T
    x2 = 2*(z @ (mask*T2).T)*x1 - T0
    x3 = 2*(z @ (mask*T3).T)*x2 - x1
    out = x3 @ C_w.T + C_b

The mask is a locally-connected conv pattern: 16x16 patch, stride 2, 25x25
positions, stacked 10x.  Rows sharing patch-row i and a patch-column BLOCK
(j-range) share a 16-row x 32-col window of the image -- 512 contraction
entries instead of the 1024-wide full-row window, which halves both the
weight bytes streamed from HBM and the matmul count vs the row-only
grouping.

Per core: 3 whole patch-row groups (i = 3c .. 3c+2), each split into three
j-blocks {0-8}, {9-16}, {17-24} (90/80/80 k-columns).  z is staged per
column band (cols [0,32), [18,50), [32,64)) as a 640-entry array of the
core's 20 image rows; group Delta=0/2 windows are 128-aligned slices (4
chunks), the Delta=1 window is covered with 5 chunks whose first/last
weight pieces are zero-padded half-chunks.  Group 24 (the 25th) is split
across cores by j-bands (3-4 positions, 352-entry window, 3 chunks).

The Chebyshev recurrence is elementwise in [k, B]; its five ops per unit
are spread over three engines (ACT: x1 copy + T0 bias add, DVE: the two
products, GPSIMD: the x3 subtract) so no single engine paces the PE.  The
k->O projection accumulates in PSUM per core; per-core partials are summed
on the host.  A burst of dummy matmuls on memset data warms the PE clock
(HAM) before the first weights arrive.
"""

import ml_dtypes
import numpy as np

import concourse.bass as bass
import concourse.mybir as mybir
import concourse.tile as tile
from concourse import bacc
from concourse.bass_utils import run_bass_kernel_spmd

F32 = mybir.dt.float32
F32R = mybir.dt.float32r

B = 256          # batch
O = 10           # output classes
D = 64           # image side
N_CORES = 8
N_GROUPS = 3     # whole patch-row groups per core
MM_MODE = "f16"
MM_DT = {"f16": mybir.dt.float16, "bf16": mybir.dt.bfloat16,
         "f32r": F32R}[MM_MODE]

# j-blocks within a group and their z column bands
BLK_J = (range(0, 9), range(9, 17), range(17, 25))
BLK_C0 = (0, 18, 32)          # band start column
BLK_COLS = (90, 80, 80)       # 10 * len(j-block)
BW = 32                       # band width
ZB_CH = 5                     # chunks per band array (20 rows * 32 = 640)
# chunk lists per group Delta (window = band entries [64*Delta, 64*Delta+512))
CHUNKS = ((0, 1, 2, 3), (0, 1, 2, 3, 4), (1, 2, 3, 4))

# mini: group 24 split by j-bands across cores
MINI_J0 = (0, 3, 6, 9, 12, 15, 18, 21)
MINI_NJ = (3, 3, 3, 3, 3, 3, 3, 4)
MINI_W = 22                   # padded window width (2*4+14)
MINI_CH = 3                   # ceil(16*22/128)
MINI_COLS = 40                # padded k-columns per core

N_UNITS = 3 * N_GROUPS + 1    # 9 full units + 1 mini
W_COLS_G = [len(CHUNKS[d]) * sum(BLK_COLS) for d in range(3)]  # 1000/1250/1000


def _build_nc():
    nc = bacc.Bacc(
        "TRN2", target_bir_lowering=False, debug=False, num_devices=N_CORES
    )
    # banded z: 3 band arrays of 5 chunks each, already chunk-blocked into
    # SBUF layout on the host so every z DMA is a plain contiguous 2D copy
    zb = nc.dram_tensor("zb", [3 * 128, ZB_CH * B], MM_DT,
                        kind="ExternalInput").ap()
    # one packed weight tensor per (layer, group) so each loads with a
    # single fully-contiguous DMA; columns in (block, piece) order
    w_dram = {
        (li, g): nc.dram_tensor(f"w{li + 1}g{g}", [128, W_COLS_G[g]], MM_DT,
                                kind="ExternalInput").ap()
        for li in range(3) for g in range(N_GROUPS)
    }
    # mini z window (3 chunks, host chunk-blocked) and weights
    zg = nc.dram_tensor("zg", [128, MINI_CH * B], MM_DT,
                        kind="ExternalInput").ap()
    wm_dram = nc.dram_tensor("wm", [128, 3 * MINI_CH * MINI_COLS], MM_DT,
                             kind="ExternalInput").ap()
    t0n = nc.dram_tensor("t0n", [128, N_UNITS], F32, kind="ExternalInput").ap()
    cwt = nc.dram_tensor("cwt", [128, N_UNITS * O], F32R,
                         kind="ExternalInput").ap()
    out = nc.dram_tensor("out", [O, B], F32, kind="ExternalOutput").ap()

    # column offset of block b inside a (layer, group) weight tensor
    def blk_off(g, b):
        return len(CHUNKS[g]) * sum(BLK_COLS[:b])

    with tile.TileContext(nc) as tc:
        with (
            tc.tile_pool(name="zpool", bufs=1) as zpool,
            tc.tile_pool(name="cpool", bufs=1) as cpool,
            tc.tile_pool(name="wpool", bufs=12) as wpool,
            tc.tile_pool(name="xpool", bufs=8) as xpool,
            tc.tile_pool(name="ppool", bufs=7, space="PSUM") as ppool,
            tc.tile_pool(name="opool", bufs=1, space="PSUM") as opool,
        ):
            engs = (nc.sync, nc.scalar)
            n_dma = 0

            def dma(dst, src):
                nonlocal n_dma
                engs[n_dma % 2].dma_start(dst, src)
                n_dma += 1

            # HAM warm-up: dummy matmuls on memset data keep the PE busy
            # from the start so its clock unthrottles while weights stream.
            dummy_sb = zpool.tile([128, B], MM_DT, tag="warm")
            nc.gpsimd.memset(dummy_sb[:], 0)
            dummy_ps = ppool.tile([128, B], F32, tag="ps")

            def warm(n):
                for _ in range(n):
                    nc.tensor.matmul(dummy_ps[:], dummy_sb[:, 0:128],
                                     dummy_sb[:], start=True, stop=True)

            warm(12)

            # z band tiles, chunk-blocked [128, ZB_CH*B]
            zbt = [zpool.tile([128, ZB_CH * B], MM_DT, tag=f"zb{b}",
                              name=f"zbt{b}")
                   for b in range(3)]

            def zb_dma(b, c0, c1, eng):
                eng.dma_start(zbt[b][:, c0 * B:c1 * B],
                              zb[b * 128:(b + 1) * 128, c0 * B:c1 * B])

            # weight tile for one (group, layer): all 3 blocks, one
            # contiguous DMA (split in two for the very first tile)
            def wload(g, li, split=False):
                t = wpool.tile([128, W_COLS_G[g]], MM_DT, tag="w")
                if split:
                    half = blk_off(g, 1)
                    nc.sync.dma_start(t[:, 0:half], w_dram[(li, g)][:, 0:half])
                    nc.sync.dma_start(t[:, half:], w_dram[(li, g)][:, half:])
                else:
                    dma(t[:], w_dram[(li, g)][:])
                return t

            # critical-path prefix: first layer-group weights on one queue,
            # its z band chunks on the other
            wt = {}
            wt[(0, 0)] = wload(0, 0, split=True)
            zb_dma(0, 0, 2, nc.scalar)
            zb_dma(0, 2, 4, nc.scalar)
            wt[(0, 1)] = wload(0, 1, split=True)
            zb_dma(1, 0, 2, nc.scalar)
            zb_dma(1, 2, 4, nc.scalar)
            wt[(0, 2)] = wload(0, 2)
            zb_dma(2, 0, 4, nc.scalar)

            t0_sb = cpool.tile([128, N_UNITS], F32, tag="t0")
            cw_sb = cpool.tile([128, N_UNITS * O], F32R, tag="cw")
            dma(t0_sb[:], t0n[:])
            dma(cw_sb[:], cwt[:])

            for li in range(3):
                wt[(1, li)] = wload(1, li)
            zb_dma(0, 4, 5, nc.sync)
            zb_dma(1, 4, 5, nc.scalar)
            zb_dma(2, 4, 5, nc.sync)

            # mini data
            zgt = zpool.tile([128, MINI_CH * B], MM_DT, tag="zg")
            nc.scalar.dma_start(zgt[:], zg[:])
            wm = wpool.tile([128, 3 * MINI_CH * MINI_COLS], MM_DT, tag="wm")
            dma(wm[:], wm_dram[:])

            psum_o = opool.tile([O, B], F32)
            n_proj = 0
            pending = []   # deferred projection matmuls (src, unit, rows)

            def project(src_t, unit, rows):
                pending.append((src_t, unit, rows))

            def flush_proj():
                nonlocal n_proj
                for src_t, unit, rows in pending:
                    n_proj += 1
                    nc.tensor.matmul(psum_o[:],
                                     cw_sb[0:rows, unit * O:(unit + 1) * O],
                                     src_t[:],
                                     start=(n_proj == 1),
                                     stop=(n_proj == N_UNITS))
                pending.clear()

            def recurrence(li, p, xs, unit, rows):
                """Chebyshev step for one unit; ops spread over ACT/DVE/GP."""
                if li == 0:
                    x1 = xpool.tile([rows, B], F32, tag="x1")
                    nc.scalar.copy(x1[:], p[:])
                    xs["x1"] = x1
                elif li == 1:
                    m2 = xpool.tile([rows, B], F32, tag="m2")
                    x2 = xpool.tile([rows, B], F32, tag="x2")
                    nc.vector.tensor_mul(m2[:], p[:], xs["x1"][:])
                    nc.scalar.add(x2[:], m2[:], t0_sb[0:rows, unit:unit + 1])
                    xs["x2"] = x2
                else:
                    m3 = xpool.tile([rows, B], F32, tag="m3")
                    x3 = xpool.tile([rows, B], F32R, tag="x3")
                    nc.vector.tensor_mul(m3[:], p[:], xs["x2"][:])
                    nc.gpsimd.tensor_sub(x3[:], m3[:], xs["x1"][:])
                    project(x3, unit, rows)

            def group(g, fill=0):
                units = [{}, {}, {}]
                chunks = CHUNKS[g]
                for li in range(3):
                    if li == 1:
                        flush_proj()
                    for b in range(3):
                        m = BLK_COLS[b]
                        w = wt[(g, li)]
                        o = blk_off(g, b)
                        p = ppool.tile([m, B], F32, tag="ps")
                        for pi, kc in enumerate(chunks):
                            nc.tensor.matmul(
                                p[:], w[:, o + pi * m:o + (pi + 1) * m],
                                zbt[b][:, kc * B:(kc + 1) * B],
                                start=(pi == 0), stop=(pi == len(chunks) - 1))
                            if li == 0:
                                warm(fill)
                        recurrence(li, p, units[b], 3 * g + b, m)

            group(0, fill=1)

            for li in range(3):
                wt[(2, li)] = wload(2, li)

            group(1, fill=1)

            # mini between the two remaining groups to cover their DMA
            mini = {}
            for li in range(3):
                if li == 1:
                    flush_proj()
                p = ppool.tile([MINI_COLS, B], F32, tag="ps")
                for kc in range(MINI_CH):
                    lhsT = wm[:, (li * MINI_CH + kc) * MINI_COLS:
                              (li * MINI_CH + kc + 1) * MINI_COLS]
                    nc.tensor.matmul(p[:], lhsT, zgt[:, kc * B:(kc + 1) * B],
                                     start=(kc == 0), stop=(kc == MINI_CH - 1))
                recurrence(li, p, mini, N_UNITS - 1, MINI_COLS)

            group(2)
            flush_proj()

            out_sb = cpool.tile([O, B], F32, tag="out")
            nc.scalar.copy(out_sb[:], psum_o[:])
            nc.sync.dma_start(out[:], out_sb[:])

    nc.compile()
    return nc


_NC = None


def _get_nc():
    global _NC
    if _NC is None:
        _NC = _build_nc()
    return _NC


def _win_idx(r0, c0, rows, width):
    """d*d indices of a rows x width window flattened row-major."""
    e = np.arange(rows * width)
    return (r0 + e // width) * D + c0 + e % width


def _prepare_in_maps(z, T1, T2, T3, T0, C_w, mask):
    z = np.ascontiguousarray(np.asarray(z, dtype=np.float32).reshape(B, D * D))
    T1 = np.asarray(T1, dtype=np.float32)
    T2 = np.asarray(T2, dtype=np.float32)
    T3 = np.asarray(T3, dtype=np.float32)
    T0 = np.asarray(T0, dtype=np.float32)
    C_w = np.asarray(C_w, dtype=np.float32)
    mask = np.asarray(mask, dtype=np.float32)

    np_mm = {"f16": np.float16, "bf16": ml_dtypes.bfloat16,
             "f32r": np.float32}[MM_MODE]
    zT = np.ascontiguousarray(z.T)                   # [4096, 256]
    A = [sc * T * mask for T, sc in ((T1, 1.0), (T2, 2.0), (T3, 2.0))]

    in_maps = []
    for c in range(N_CORES):
        i0 = 3 * c
        r0 = 2 * i0
        m = {}

        # banded z: per band, 640 entries = 20 rows x 32 cols, chunk-blocked
        # into SBUF layout [128, ZB_CH*B]
        zb = np.empty((3 * 128, ZB_CH * B), np.float32)
        for b in range(3):
            idx = _win_idx(r0, BLK_C0[b], 20, BW)
            zb[b * 128:(b + 1) * 128] = (
                zT[idx].reshape(ZB_CH, 128, B).transpose(1, 0, 2)
                .reshape(128, ZB_CH * B))
        m["zb"] = np.ascontiguousarray(zb).astype(np_mm)

        # weights: one tensor per (layer, group), pieces in (block, piece)
        # column order so the kernel loads each with one contiguous DMA
        t0n = np.zeros((128, N_UNITS), np.float32)
        cwt = np.zeros((128, N_UNITS * O), np.float32)
        for l in range(3):
            for g in range(N_GROUPS):
                gg = i0 + g
                chunks = CHUNKS[g]
                wl = np.zeros((128, W_COLS_G[g]), np.float32)
                col = 0
                for b in range(3):
                    cols_k = np.array([s * 625 + gg * 25 + j
                                       for s in range(10) for j in BLK_J[b]])
                    win = _win_idx(2 * gg, BLK_C0[b], 16, BW)
                    AT = A[l][np.ix_(cols_k, win)].T      # [512, m]
                    mcols = len(cols_k)
                    for pi, kc in enumerate(chunks):
                        piece = np.zeros((128, mcols), np.float32)
                        # band entries of chunk kc vs window [64g, 64g+512)
                        lo = kc * 128 - 64 * g
                        wlo, whi = max(lo, 0), min(lo + 128, 512)
                        piece[wlo - lo:whi - lo] = AT[wlo:whi]
                        wl[:, col:col + mcols] = piece
                        col += mcols
                    if l == 0:
                        u = 3 * g + b
                        t0n[0:mcols, u] = -T0[cols_k]
                        cwt[0:mcols, u * O:(u + 1) * O] = C_w[:, cols_k].T
                m[f"w{l + 1}g{g}"] = np.ascontiguousarray(wl).astype(np_mm)

        # mini: group 24, this core's j-band
        j0, nj = MINI_J0[c], MINI_NJ[c]
        mcols_k = np.array([s * 625 + 600 + j
                            for s in range(10) for j in range(j0, j0 + nj)])
        nmc = len(mcols_k)
        gwin = _win_idx(48, 2 * j0, 16, MINI_W)          # 352 entries
        zgm = np.zeros((MINI_CH * 128, B), np.float32)
        zgm[0:352] = zT[gwin]
        m["zg"] = np.ascontiguousarray(
            zgm.reshape(MINI_CH, 128, B).transpose(1, 0, 2)
            .reshape(128, MINI_CH * B)).astype(np_mm)
        wm = np.zeros((128, 3 * MINI_CH * MINI_COLS), np.float32)
        for l in range(3):
            AT = A[l][np.ix_(mcols_k, gwin)].T           # [352, nmc]
            for kc in range(MINI_CH):
                piece = np.zeros((128, MINI_COLS), np.float32)
                hi = min(352 - kc * 128, 128)
                piece[0:hi, 0:nmc] = AT[kc * 128:kc * 128 + hi]
                wm[:, (l * MINI_CH + kc) * MINI_COLS:
                   (l * MINI_CH + kc + 1) * MINI_COLS] = piece
        m["wm"] = wm.astype(np_mm)
        u = N_UNITS - 1
        t0n[0:nmc, u] = -T0[mcols_k]
        cwt[0:nmc, u * O:(u + 1) * O] = C_w[:, mcols_k].T
        m["t0n"] = t0n
        m["cwt"] = cwt
        in_maps.append(m)
    return in_maps


def kernel(z, T1, T2, T3, T0, C_w, C_b, mask):
    nc = _get_nc()
    in_maps = _prepare_in_maps(z, T1, T2, T3, T0, C_w, mask)
    res = run_bass_kernel_spmd(nc, in_maps, core_ids=list(range(N_CORES)))
    total = np.zeros((O, B), np.float32)
    for c in range(N_CORES):
        total += res.results[c]["out"]
    C_b = np.asarray(C_b, dtype=np.float32)
    return (total.T + C_b).astype(np.float32)
